# revision 1
# baseline (speedup 1.0000x reference)
"""Trainium2 Bass kernel for nn_PairwiseAttentionTerminal — redesigned.

Cost-model-driven redesign (CoreSim charges: matmul = out-free-rows x cpr;
LDWEIGHTS free; ACT/DVE = free + access-penalty; Pool = free-size, SBUF only;
DMA charged to issuing queue):

  - AV flipped to q-major: exp(S^T) tiles become the STATIONARY matmul
    operand, v (32-col, bf16) moves -> 512 MMs x 32 rows (16.4k PE cycles)
    vs 65.5k k-major.  Denominators via 512 free-1 MMs against a ones column.
  - exp(S) split ACT/DVE: ACT native Exp; DVE computes a bf16 Schraudolph
    bit-trick exp: i16 = round(A*S + (A*bias + B)) reinterpreted as bf16.
  - All q/k biases folded away (softmax shift-invariance) into the per-key
    bias projection; LN gamma/beta folded into every projection weight.
  - sigmoid gate via exp + add1(Pool) + native DVE reciprocal; all transposes
    via DMA-transpose (bf16, SBUF->SBUF, off-engine); LN stats on GPSIMD.
  - v bias enters post-normalize (sum w (v+bv) = sum w v + bv).
  - weights shipped bf16; everything bf16 on-chip except PSUM accumulators.

Sharding: batch B=8 -> one batch element per core, weights replicated.
"""

import numpy as np
from contextlib import ExitStack

L, B, F, H, C = 1024, 8, 256, 8, 32
HC = H * C
EPS = 1e-5
N_CORES = 8
P = 128
NLT = L // P   # 8 L-tiles
NFC = F // P   # 2 F-chunks
NHC = HC // P  # 2 hc-chunks

A_SCH = float(128.0 / np.log(2.0))
B_SCH = float((127.0 - 0.043) * 128.0)
ACT_EXP = 31  # of 64 attention exp tiles on ACT (rest: DVE Schraudolph)

_COMPILED = {}


def _build():
    import concourse.bacc as bacc
    import concourse.mybir as mybir
    import concourse.tile as tile

    f32 = mybir.dt.float32
    f32r = mybir.dt.float32r
    bf16 = mybir.dt.bfloat16
    i16 = mybir.dt.int16
    AF = mybir.ActivationFunctionType
    ALU = mybir.AluOpType

    nc = bacc.Bacc("TRN2", target_bir_lowering=False)

    feat_e = nc.dram_tensor("feat", [L, F], f32, kind="ExternalInput")
    wq_e = nc.dram_tensor("wq", [P, NFC, HC], bf16, kind="ExternalInput")
    wk_e = nc.dram_tensor("wk", [P, NFC, HC], bf16, kind="ExternalInput")
    wv_e = nc.dram_tensor("wv", [P, NFC, HC], bf16, kind="ExternalInput")
    wg_e = nc.dram_tensor("wg", [P, NFC, HC], bf16, kind="ExternalInput")
    wb_e = nc.dram_tensor("wb", [P, NFC, H], bf16, kind="ExternalInput")
    wo_e = nc.dram_tensor("wo", [P, NHC, F], bf16, kind="ExternalInput")
    nbg_e = nc.dram_tensor("nbg", [P, NHC], f32, kind="ExternalInput")
    bbb_e = nc.dram_tensor("bbb", [P, H], f32, kind="ExternalInput")
    bvb_e = nc.dram_tensor("bvb", [P, HC], bf16, kind="ExternalInput")
    bob_e = nc.dram_tensor("bob", [1, 2 * F], bf16, kind="ExternalInput")
    idb_e = nc.dram_tensor("idb", [P, P], bf16, kind="ExternalInput")
    out_e = nc.dram_tensor("out", [L, F], f32, kind="ExternalOutput")

    with tile.TileContext(nc) as tc, ExitStack() as ctx:
        const = ctx.enter_context(tc.tile_pool(name="const", bufs=1))
        main = ctx.enter_context(tc.tile_pool(name="main", bufs=1))
        work = ctx.enter_context(tc.tile_pool(name="work", bufs=4))
        epool = ctx.enter_context(tc.tile_pool(name="epool", bufs=12))
        opool = ctx.enter_context(tc.tile_pool(name="opool", bufs=4))

        # ---- input DMAs (feat chunked across SP/ACT/Pool queues) ----
        ftp = ctx.enter_context(tc.tile_pool(name="ftp", bufs=1))
        ftall = ftp.tile([P, NLT, F], f32, name="ftall")
        ft = [ftall[:, i, :] for i in range(NLT)]
        fview = feat_e.ap().rearrange("(i p) f -> p i f", p=P)
        nc.sync.dma_start(ftall[:, 0:1, :], fview[:, 0:1, :])
        nc.gpsimd.dma_start(ftall[:, 4:6, :], fview[:, 4:6, :])
        nc.sync.dma_start(ftall[:, 1:4, :], fview[:, 1:4, :])
        nc.gpsimd.dma_start(ftall[:, 6:8, :], fview[:, 6:8, :])

        def load(name, ext, shape, dt_):
            t = const.tile(shape, dt_, name=name)
            nc.sync.dma_start(t[:], ext.ap())
            return t

        identb = load("idb_s", idb_e, [P, P], bf16)
        wq = load("wq_s", wq_e, [P, NFC, HC], bf16)
        wk = load("wk_s", wk_e, [P, NFC, HC], bf16)
        wb = load("wb_s", wb_e, [P, NFC, H], bf16)
        nbg = load("nbg_s", nbg_e, [P, NHC], f32)
        bbb = load("bbb_s", bbb_e, [P, H], f32)
        epst = const.tile([P, 1], f32, name="epst")
        nc.vector.memset(epst[:], EPS)
        onesf = const.tile([1, P], bf16, name="onesf")
        nc.vector.memset(onesf[:], 1.0)



        # ---- persistent SBUF ----
        xn = [main.tile([P, F], bf16, name=f"xn{i}") for i in range(NLT)]
        xT = [main.tile([P, L], bf16, name=f"xT{j}") for j in range(NFC)]
        qTs = [main.tile([P, L], bf16, name=f"qT{j}") for j in range(NHC)]
        kTs = [main.tile([P, L], bf16, name=f"kT{j}") for j in range(NHC)]
        gTb = [main.tile([P, L], bf16, name=f"gTb{j}") for j in range(NHC)]
        gateall = main.tile([P, NLT, HC], bf16, name="gateall")
        vaug = main.tile([P, NLT, HC + 1], bf16, name="vaug")
        nc.vector.memset(vaug[:, :, HC], 1.0)
        bTsb = main.tile([P, NLT * H], f32, name="bTsb")
        sbT = main.tile([P, NLT * H], f32, name="sbT")
        agall = main.tile([P, NLT, HC], bf16, name="agall")
        agT = [main.tile([P, L], bf16, name=f"agT{j}") for j in range(NHC)]
        stat = main.tile([P, 48], f32, name="stat")

        psT_cm = tc.tile_pool(name="psT", bufs=2, space="PSUM")
        psT = psT_cm.__enter__()

        # ======= Stage A: LN (bn_stats) + PE transpose, 2 batches ===
        # stat cols: [0:16] (mean,var) interleaved per tile; [16:24] rstd
        _batches = [1, 1, 2, 4]
        _start = 0
        for _b, _n in enumerate(_batches):
            bns = work.tile([P, 8, 6], f32, tag="bns")
            for t_ in range(_n):
                i = _start + t_
                nc.vector.bn_stats(bns[:, t_, :], ft[i])
                nc.vector.bn_aggr(stat[:, 2 * i:2 * i + 2], bns[:, t_, :])
            var_ap = (stat[:, 2 * _start:2 * (_start + _n)]
                      .rearrange("p (i t) -> p i t", t=2)[:, :, 1])
            rs = stat[:, 16 + _start:16 + _start + _n]
            nc.scalar.activation(rs, var_ap, AF.Ln, bias=epst[:])
            nc.scalar.activation(rs, rs, AF.Exp, scale=-0.5)
            for t_ in range(_n):
                i = _start + t_
                g = 0 if i < 4 else 1
                nc.gpsimd.tensor_scalar(xn[i][:], ft[i], stat[:, 2 * i:2 * i + 1],
                                  stat[:, 16 + i:17 + i],
                                  op0=ALU.subtract, op1=ALU.mult)
                if g == 0:
                    # early tiles: PE transpose + copy (short latency, warms PE)
                    tp = psT.tile([P, 256], bf16, tag="t", name=f"tp{i}")
                    for j in range(NFC):
                        nc.tensor.transpose(tp[:, j * P:(j + 1) * P],
                                            xn[i][:, j * P:(j + 1) * P],
                                            identb[:])
                        nc.scalar.activation(xT[j][:, i * P:(i + 1) * P],
                                             tp[:, j * P:(j + 1) * P],
                                             AF.Copy)
                else:
                    # late tiles: DMA transpose (latency hides behind m=0 work,
                    # keeps PE queue free and ACT/DVE unloaded)
                    for j in range(NFC):
                        nc.sync.dma_start_transpose(
                            xT[j][:, i * P:(i + 1) * P],
                            xn[i][:, j * P:(j + 1) * P])
            _start += _n

        # late weights on SP after the stage-A work is queued
        wv = load("wv_s", wv_e, [P, NFC, HC], bf16)
        wg = load("wg_s", wg_e, [P, NFC, HC], bf16)
        wo = load("wo_s", wo_e, [P, NHC, F], bf16)
        bvb = load("bvb_s", bvb_e, [P, HC], bf16)
        bob = load("bob_s", bob_e, [1, 2 * F], bf16)

        psT_cm.__exit__(None, None, None)

        # ================= Stage B: projections =================
        # chunk-0 q/k, per-key bias, and v are issued up front; chunk-1 q/k
        # and the gate are deferred into the attention stream (their PSUM
        # comes from the psS pool) so attention starts ~5us earlier.
        psP_cm = tc.tile_pool(name="psP", bufs=2, space="PSUM")
        psP = psP_cm.__enter__()

        def proj_qk(w_, dst, j, eng, pool, tag):
            ps = pool.tile([P, L], f32, tag=tag, name=f"pqk{dst[j].name}{j}")
            for m in range(2):
                ms = slice(512 * m, 512 * (m + 1))
                for jj in range(NFC):
                    nc.tensor.matmul(ps[:, ms],
                                     w_[:, jj, j * P:(j + 1) * P],
                                     xT[jj][:, ms],
                                     start=(jj == 0), stop=(jj == 1))
            if eng == "act":
                nc.scalar.activation(dst[j][:], ps[:], AF.Copy)
            else:
                nc.vector.tensor_copy(dst[j][:], ps[:])

        def proj_gate(j, pool, tag):
            ps = pool.tile([P, L], f32, tag=tag, name=f"pg{j}")
            for m in range(2):
                ms = slice(512 * m, 512 * (m + 1))
                for jj in range(NFC):
                    nc.tensor.matmul(ps[:, ms], wg[:, jj, j * P:(j + 1) * P],
                                     xT[jj][:, ms],
                                     start=(jj == 0), stop=(jj == 1))
            egf = work.tile([P, L], f32, tag="egf", name=f"egf{j}")
            nc.scalar.activation(egf[:], ps[:], AF.Exp, scale=-1.0,
                                 bias=nbg[:, j:j + 1])
            nc.gpsimd.tensor_scalar(egf[:], egf[:], 1.0, None, op0=ALU.add)
            grec = work.tile([P, L], f32, tag="grec", name=f"grec{j}")
            nc.vector.reciprocal(grec[:], egf[:])
            nc.gpsimd.tensor_copy(gTb[j][:], grec[:])
            for i in range(NLT):
                nc.sync.dma_start_transpose(
                    gateall[:, i, j * P:(j + 1) * P],
                    gTb[j][:, i * P:(i + 1) * P])

        proj_qk(wq, qTs, 0, "act", psP, "p")
        proj_qk(wk, kTs, 0, "act", psP, "p")

        vview = vaug[:]

        def proj_v(pr):
            ps = psS.tile([P, 512], f32, tag="s", name=f"pv{pr}")
            for t_ in range(2):
                i = 2 * pr + t_
                for jj in range(NFC):
                    nc.tensor.matmul(ps[:, t_ * 256:(t_ + 1) * 256],
                                     xT[jj][:, i * P:(i + 1) * P],
                                     wv[:, jj, :],
                                     start=(jj == 0), stop=(jj == 1))
            dst = vview[:, 2 * pr:2 * pr + 2, 0:HC]
            src = ps[:].rearrange("p (t c) -> p t c", t=2)
            nc.scalar.activation(dst, src, AF.Copy)

        # per-key bias projection (all 8 L-tiles into one PSUM bank)
        psB = psP.tile([P, 64], f32, tag="p", name="pb")
        for i in range(NLT):
            for jj in range(NFC):
                nc.tensor.matmul(psB[:, i * H:(i + 1) * H],
                                 xT[jj][:, i * P:(i + 1) * P],
                                 wb[:, jj, :], start=(jj == 0), stop=(jj == 1))
        nc.vector.tensor_tensor(
            bTsb[:].rearrange("p (k h) -> p k h", k=NLT),
            psB[:].rearrange("p (k h) -> p k h", k=NLT),
            bbb[:].unsqueeze(1).broadcast_to([P, NLT, H]), op=ALU.add)
        nc.gpsimd.tensor_scalar(sbT[:], bTsb[:], A_SCH, B_SCH,
                                op0=ALU.mult, op1=ALU.add)

        psP_cm.__exit__(None, None, None)

        # ================= Stage C: attention =================
        # Heads processed in PAIRS: psA = 1 bank [8qt x 2h x 32], psD = 1 bank
        # -> psS gets 3 buffers (6 banks) for a deep QK->exp pipeline.
        psA_cm = tc.tile_pool(name="psA", bufs=1, space="PSUM")
        psA = psA_cm.__enter__()
        psD_cm = tc.tile_pool(name="psD", bufs=1, space="PSUM")
        psDp = psD_cm.__enter__()
        psS_cm = tc.tile_pool(name="psS", bufs=3, space="PSUM")
        psS = psS_cm.__enter__()

        # Bresenham split of the 64 exp tiles onto ACT
        act_tile, acc = {}, 0
        for t_ in range(64):
            acc += ACT_EXP
            act_tile[t_] = acc >= 64
            if acc >= 64:
                acc -= 64

        psA_t = {}
        psD_t = {}
        eT = {}

        def issue_av(h, kk):
            pr = h // 2
            if pr not in psA_t:
                psA_t[pr] = psA.tile([P, 512], f32, tag="a", name=f"pa{pr}")
                psD_t[pr] = psDp.tile([P, 16], f32, tag="d", name=f"pd{pr}")
            pa, pd = psA_t[pr], psD_t[pr]
            e = eT[(h, kk)]
            first = (kk == 0 and h % 2 == 0)
            last = (kk == NLT - 1 and h % 2 == 1)
            for qt in range(NLT):
                lhs = e[:, qt * P:(qt + 1) * P]
                c0 = qt * 64 + (h % 2) * C
                nc.tensor.matmul(pa[:, c0:c0 + C], lhs,
                                 vaug[:, kk, h * C:(h + 1) * C],
                                 start=(first and qt == 0),
                                 stop=(last and qt == NLT - 1))
                dcol = qt * 2 + (h % 2)
                nc.tensor.matmul(pd[:, dcol:dcol + 1], lhs,
                                 vaug[:, kk, HC:HC + 1],
                                 start=(first and qt == 0),
                                 stop=(last and qt == NLT - 1))

        def drain_pair(pr, fine=False):
            """normalize + bias + gate for heads (2pr, 2pr+1)."""
            pa, pd = psA_t[pr], psD_t[pr]
            dsb = work.tile([P, 16], f32, tag="dsb", name=f"dsb{pr}")
            nc.vector.tensor_copy(dsb[:], pd[:])
            rec = work.tile([P, 16], f32, tag="rec", name=f"rec{pr}")
            nc.vector.reciprocal(rec[:], dsb[:])
            cs = slice(pr * 64, (pr + 1) * 64)
            steps = [range(NLT)] if not fine else [range(2 * t, 2 * t + 2)
                                                  for t in range(4)]
            for qts in steps:
                q0, q1 = qts[0], qts[-1] + 1
                agv = agall[:, q0:q1, cs]
                nc.vector.tensor_tensor(
                    agv.rearrange("p q (h c) -> p q h c", h=2),
                    pa[:].rearrange("p (q hc) -> p q hc", q=NLT)
                    [:, q0:q1, :].rearrange("p q (h c) -> p q h c", h=2),
                    rec[:].rearrange("p (q h) -> p q h", q=NLT)[:, q0:q1, :]
                    .unsqueeze(3).broadcast_to([P, q1 - q0, 2, C]),
                    op=ALU.mult)
                nc.gpsimd.tensor_tensor(
                    agv, agv,
                    bvb[:, cs].unsqueeze(1).broadcast_to([P, q1 - q0, 64]),
                    op=ALU.add)
                nc.gpsimd.tensor_tensor(agv, agv, gateall[:, q0:q1, cs],
                                        op=ALU.mult)
                if pr == 1:
                    for qt in qts:
                        nc.sync.dma_start_transpose(
                            agT[0][:, qt * P:(qt + 1) * P],
                            agall[:, qt, 0:P])
                if fine:
                    # last agT chunk via PE transpose (no DMA latency) and
                    # the output projection right behind it
                    tp = psS.tile([P, 256], bf16, tag="s", name=f"tp{q0}")
                    for t_, qt in enumerate(qts):
                        nc.tensor.transpose(tp[:, t_ * P:(t_ + 1) * P],
                                            agall[:, qt, P:2 * P], identb[:])
                        nc.vector.tensor_copy(
                            agT[1][:, qt * P:(qt + 1) * P],
                            tp[:, t_ * P:(t_ + 1) * P])
                    out_pair(q0 // 2)

        def out_pair(pr2):
            ps = psS.tile([P, 512], f32, tag="s", name=f"po{pr2}")
            for t_ in range(2):
                i = 2 * pr2 + t_
                osl = ps[:, t_ * 256:(t_ + 1) * 256]
                nc.tensor.matmul(osl, onesf[:],
                                 bob[:, t_ * 256:(t_ + 1) * 256],
                                 start=True, stop=False)
                for j in range(NHC):
                    nc.tensor.matmul(osl, agT[j][:, i * P:(i + 1) * P],
                                     wo[:, j, :], start=False, stop=(j == 1))
            o = opool.tile([P, 512], f32, tag="o", name=f"ot{pr2}")
            nc.scalar.activation(o[:], ps[:], AF.Copy)
            for t_ in range(2):
                i = 2 * pr2 + t_
                eng = nc.gpsimd if t_ == 0 else nc.sync
                eng.dma_start(out_e.ap()[i * P:(i + 1) * P, :],
                              o[:, t_ * 256:(t_ + 1) * 256])

        deferred = {
            (0, 0): lambda: proj_v(0),
            (0, 2): lambda: proj_v(1),
            (0, 4): lambda: proj_v(2),
            (0, 6): lambda: proj_v(3),
            (0, 1): lambda: proj_qk(wq, qTs, 1, "act", psS, "s"),
            (0, 3): lambda: proj_qk(wk, kTs, 1, "act", psS, "s"),
            (0, 7): lambda: proj_gate(0, psS, "s"),
            (1, 3): lambda: proj_gate(1, psS, "s"),
        }
        prev = None
        for h in range(H):
            jh, ph = h // 4, 32 * (h % 4)
            hp = slice(ph, ph + 32)
            for kk in range(NLT):
                if (h, kk) in deferred:
                    deferred[(h, kk)]()
                sp = psS.tile([P, L], f32, tag="s", name=f"sp{h}_{kk}")
                for m in range(2):
                    ms = slice(512 * m, 512 * (m + 1))
                    nc.tensor.matmul(sp[:, ms],
                                     kTs[jh][hp, kk * P:(kk + 1) * P],
                                     qTs[jh][hp, ms], start=True, stop=True,
                                     tile_position=(ph, 0))
                e = epool.tile([P, L], bf16, tag="e", name=f"e{h}_{kk}")
                bcol = kk * H + h
                if act_tile[h * NLT + kk]:
                    nc.scalar.activation(e[:], sp[:], AF.Exp,
                                         bias=bTsb[:, bcol:bcol + 1])
                else:
                    nc.vector.tensor_scalar(e[:].bitcast(i16), sp[:], A_SCH,
                                            sbT[:, bcol:bcol + 1],
                                            op0=ALU.mult, op1=ALU.add)
                eT[(h, kk)] = e
                if prev is not None:
                    issue_av(*prev)
                    if prev[0] % 2 == 1 and prev[1] == NLT - 1:
                        drain_pair(prev[0] // 2)
                prev = (h, kk)
        issue_av(*prev)
        drain_pair(H // 2 - 1, fine=True)

        psS_cm.__exit__(None, None, None)
        psD_cm.__exit__(None, None, None)
        psA_cm.__exit__(None, None, None)

    # Pin Exp/Ln to the one combined table set (avoids chooser thrash).
    import concourse.bacc as bacc_mod
    orig_gat = bacc_mod.get_activation_tables

    def gat_combined(arch):
        t = orig_gat(arch)
        # single candidate set -> exactly one table load
        return {name: (funcs if name == "natural_log_exp_and_others" else set())
                for name, funcs in t.items()}

    bacc_mod.get_activation_tables = gat_combined
    try:
        nc.compile()
    finally:
        bacc_mod.get_activation_tables = orig_gat
    return nc


def _prep_inputs(features, ln_g, ln_b, Wq, bq, Wk, bk, Wv, bv, Wb, bb,
                 Wg, bg, Wo, bo):
    import ml_dtypes
    bf = ml_dtypes.bfloat16
    f32 = np.float32
    sq = f32(1.0 / np.sqrt(C))
    g_ = np.asarray(ln_g, f32)[:, None]
    beta = np.asarray(ln_b, f32)

    Wq_ = np.asarray(Wq, f32) * g_ * sq
    Wk_ = np.asarray(Wk, f32) * g_
    Wv_ = np.asarray(Wv, f32) * g_
    Wg_ = np.asarray(Wg, f32) * g_
    bq_t = (beta @ np.asarray(Wq, f32) + np.asarray(bq, f32)) * sq  # [HC]
    bv_ = beta @ np.asarray(Wv, f32) + np.asarray(bv, f32)
    bg_ = beta @ np.asarray(Wg, f32) + np.asarray(bg, f32)
    # per-key bias: Wb fold + q-bias cross term (softmax-invariant parts drop)
    WB = np.asarray(Wb, f32) * g_
    for h in range(H):
        WB[:, h] += Wk_[:, C * h:C * (h + 1)] @ bq_t[C * h:C * (h + 1)]
    BB = beta @ np.asarray(Wb, f32) + np.asarray(bb, f32)  # [H]

    def wsplit(W, n, dt_):
        return np.ascontiguousarray(
            np.asarray(W, f32).reshape(NFC, P, n).transpose(1, 0, 2)).astype(dt_)

    common = {
        "wq": wsplit(Wq_, HC, bf),
        "wk": wsplit(Wk_, HC, bf),
        "wv": wsplit(Wv_, HC, bf),
        "wg": wsplit(Wg_, HC, bf),
        "wb": wsplit(WB, H, bf),
        "wo": wsplit(np.asarray(Wo, f32), F, bf),
        "nbg": np.ascontiguousarray((-bg_).reshape(NHC, P).T).astype(f32),
        "bbb": np.ascontiguousarray(np.tile(BB, (P, 1))).astype(f32),
        "bvb": np.ascontiguousarray(np.tile(bv_, (P, 1))).astype(bf),
        "idb": np.eye(P, dtype=np.float32).astype(bf),
        "bob": np.ascontiguousarray(
            np.tile(np.asarray(bo, f32), (1, 2))).astype(bf),
    }
    feats = np.asarray(features, f32)
    in_maps = []
    for b_ in range(N_CORES):
        m = dict(common)
        m["feat"] = np.ascontiguousarray(feats[:, b_, :])
        in_maps.append(m)
    return in_maps


def kernel(**inputs):
    from concourse.bass_utils import run_bass_kernel_spmd

    if "nc" not in _COMPILED:
        _COMPILED["nc"] = _build()
    nc = _COMPILED["nc"]
    in_maps = _prep_inputs(**inputs)
    res = run_bass_kernel_spmd(nc, in_maps, list(range(N_CORES)))
    out = np.stack([res.results[b_]["out"] for b_ in range(N_CORES)], axis=1)
    return np.ascontiguousarray(out.astype(np.float32))

